# revision 8
# baseline (speedup 1.0000x reference)
"""Trainium2 Bass kernel for nn_CatEmbedder (gnn_message_passing).

Strategy (v2):
- Batch-parallel across 8 cores (4096 samples each), table replicated.
- Gather via dma_gather (one SWDGE instruction per range-pass per block)
  instead of 1600 per-field indirect DMAs (which serialized ~1.8ms on Q7).
  dma_gather needs int16 indices, so the 100k-row table is split into 4
  quarters of 25001 rows (each with a trailing zero row for padding) and
  each block issues 4 gathers. Per-sample quarter-counts vary, so samples
  are globally sorted by count profile and dealt round-robin to cores;
  each block pads every sample to the block's per-quarter max with
  zero-row lookups. Padded slots contribute relu(t_s) to the global
  branch, which is subtracted exactly via a per-sample pad-count
  correction term.
- fp16 data path everywhere (4x faster matmuls than fp32, half the
  gather bytes); fp32 PSUM accumulation. rel err ~4e-4 vs 2e-2 budget.
- Algebraic fold: gacc_f = (e_f + summed/PROBE) @ (ga_W*PROBE/CD) + ga_b,
  so the per-field bias becomes one broadcast add on the embeddings.
"""

import sys
import types
import inspect

import numpy as np

sys.path.insert(0, "/opt/trn_rl_repo")

# ---- problem constants ----
B, F, D, NCT = 32768, 50, 64, 100000
PROBE, ALPHA = 39.0, 0.5
NF = F + 1
CD = NF + PROBE
NCORES = 8
BS = B // NCORES          # 4096
BLK = 128
NBLK = BS // BLK          # 32
NQ = 4
QS = NCT // NQ            # 25000
QR = QS + 1               # 25001 rows per on-device quarter (incl zero row)

# dma_gather index layout: "interp" = idx i at [i%16, i//16];
# "bench" = idx i at [i//(n/16), i%(n/16)] — set after HW probing.
IDX_LAYOUT = "interp"

_CACHE = {}


def _patch_dma_gather():
    """Allow 128B (fp16 row) elements: the stride field is 256B-units so
    the table rows are padded to 256B stride, but the payload read per
    descriptor is 128B."""
    import concourse.bass as Bs

    if getattr(Bs.BassGpSimd.dma_gather, "_patched_128b", False):
        return
    src = inspect.getsource(Bs.BassGpSimd.dma_gather)
    src = src.replace(
        "assert (\n            elem_size_bytes > 0 and elem_size_bytes % 256 == 0\n"
        "        )",
        "assert elem_size_bytes > 0")
    src = "def dma_gather" + src.split("def dma_gather", 1)[1]
    ns = dict(Bs.__dict__)
    exec(compile(src, "<dma_gather_128b>", "exec"), ns)
    ns["dma_gather"]._patched_128b = True
    Bs.BassGpSimd.dma_gather = ns["dma_gather"]


def _wrap_idx(flat):
    """flat [n] int16 (position order i -> partition i%128, slot i//128)
    -> wrapped [128, n/16] per the SWDGE firmware layout (16-partition
    block replicated to all 8 Q7 cores' partition groups)."""
    n = flat.shape[0]
    assert n % 16 == 0
    c = n // 16
    if IDX_LAYOUT == "interp":
        w = flat.reshape(c, 16).T          # [16, c], i at [i%16, i//16]
    else:
        w = flat.reshape(16, c)            # [16, c], i at [i//c, i%c]
    return np.ascontiguousarray(np.tile(w, (8, 1)))


def _prepare(inputs):
    """Host prep: sort+deal samples, build per-block pass schedules,
    pack int16 index arrays, aux (numf / -npad), consts. Returns
    (in_maps, sched, order)."""
    import ml_dtypes  # noqa: F401

    f = np.float32
    idx = np.asarray(inputs["cat_indices"]).astype(np.int64)
    numf = np.asarray(inputs["num_features"]).astype(f)
    table = np.asarray(inputs["embed_table"]).astype(f)

    # sorted per-sample indices (fields are exchangeable), quarter counts
    idx_s = np.sort(idx, axis=1)
    qq = idx_s // QS
    c = np.stack([(qq == k).sum(1) for k in range(NQ)], 1)      # [B, 4]
    cum = np.concatenate([np.zeros((B, 1), np.int64), np.cumsum(c, 1)], 1)

    order = np.lexsort((-c[:, 1], -c[:, 0]))                     # [B]

    # per-window (= per-block-index, shared by all cores) quarter maxes
    cw = c[order].reshape(NBLK, NCORES * BLK, NQ)
    K_qb = cw.max(1)                                             # [NBLK, 4]
    for b in range(NBLK):
        if K_qb[b].sum() % 2:
            K_qb[b, 3] += 1

    # device table: quarters of 25001 rows (last row zero), fp16,
    # rows padded to 128 fp16 (256B stride; only cols 0:64 are read)
    t16 = np.zeros((NQ * QR, 128), np.float16)
    for k in range(NQ):
        t16[k * QR:k * QR + QS, 0:64] = table[k * QS:(k + 1) * QS]

    # pack per-(block, pass) index arrays for each core
    sched = []
    tot16 = 0
    for b in range(NBLK):
        Ks = [int(K_qb[b, k]) for k in range(NQ)]
        S = sum(Ks)
        sched.append((Ks, S, S // 2, tot16))
        tot16 += S * 8
    idx16 = np.empty((NCORES, 128, tot16), np.int16)
    aux = np.zeros((NCORES, NBLK, 1, 256), np.float16)

    ar = np.arange(BLK)
    for b in range(NBLK):
        Ks, S, n_c, off = sched[b]
        w = order[b * NCORES * BLK:(b + 1) * NCORES * BLK]       # [1024]
        wc = w.reshape(BLK, NCORES)                              # [p, core]
        coff = off
        for k in range(NQ):
            Kq = Ks[k]
            if Kq == 0:
                continue
            kk = np.arange(Kq)
            # vals [p, core, Kq]
            pos = cum[wc, k][:, :, None] + kk[None, None, :]
            valid = kk[None, None, :] < c[wc, k][:, :, None]
            vals = np.take_along_axis(
                idx_s[wc], np.minimum(pos, F - 1), axis=2)
            vals = np.where(valid, vals - k * QS, QS).astype(np.int16)
            for core in range(NCORES):
                flat = vals[:, core, :].T.ravel()                # i = kk*128+p
                idx16[core, :, coff:coff + Kq * 8] = _wrap_idx(flat)
            coff += Kq * 8
        npad = (np.array(Ks)[None, None, :] - c[wc]).sum(2)      # [p, core]
        for core in range(NCORES):
            aux[core, b, 0, 0:128] = numf[wc[ar, core]]
            aux[core, b, 0, 128:256] = -npad[:, core].astype(f)

    consts = _make_consts(inputs)
    in_maps = []
    for core in range(NCORES):
        m = dict(consts)
        m["table"] = t16
        m["idx16"] = np.ascontiguousarray(idx16[core])
        m["aux"] = np.ascontiguousarray(aux[core])
        in_maps.append(m)
    return in_maps, sched, order


def _make_consts(inputs):
    f = np.float32
    h = np.float16
    ga_W = np.asarray(inputs["ga_W"]).astype(f)
    ga_b = np.asarray(inputs["ga_b"]).astype(f)
    gW = np.asarray(inputs["gW"]).astype(f)
    gb = np.asarray(inputs["gb"]).astype(f)
    lW = np.asarray(inputs["lW"]).astype(f)
    lb = np.asarray(inputs["lb"]).astype(f)
    num_W = np.asarray(inputs["num_W"]).astype(f)
    num_b = np.asarray(inputs["num_b"]).astype(f)

    W1 = ga_W * (PROBE / CD)
    # carrier bias c with c @ W1 == ga_b (exact 0 when ga_b == 0)
    cvec = np.linalg.lstsq(W1.T, ga_b, rcond=None)[0]
    gw2 = np.zeros((128, 128), h)
    gw2[0:64, 0:64] = W1.astype(h)
    gw2[64:128, 64:128] = W1.astype(h)
    dup64 = np.zeros((64, 128), h)
    dup64[np.arange(64), np.arange(64)] = 1
    dup64[np.arange(64), 64 + np.arange(64)] = 1
    i64 = np.eye(64, dtype=f)
    segf = np.vstack([np.eye(64), np.eye(64)]).astype(h)
    g0t = (gW[0] / NF).T.astype(h)
    gseg = np.vstack([g0t, g0t])
    g1aug = np.zeros((65, 64), h)
    g1aug[0:64] = (ALPHA * gW[1].T).astype(h)
    g1aug[64] = (ALPHA * gb[1]).astype(h)
    l0T = ((0.5 * lW[0]).T).astype(h)
    l1aug = np.zeros((65, 64), h)
    l1aug[0:64] = ((1 - ALPHA) * lW[1].T).astype(h)
    l1aug[64] = ((1 - ALPHA) * lb[1]).astype(h)
    cols = np.stack(
        [num_W[:, 0], num_b, gb[0], lb[0], cvec], axis=1).astype(f)
    return {
        "gw2": gw2, "dup64": dup64, "segf": segf, "gseg": gseg,
        "g0t64": np.ascontiguousarray(g0t), "g1aug": g1aug,
        "l0T": l0T, "l1aug": l1aug, "i64f": i64,
        "ident": np.eye(128, dtype=h), "ones164": np.ones((1, 64), h),
        "cols": cols,
    }


def _build(sched, tot16):
    import concourse.bass as bass
    import concourse.mybir as mybir
    import concourse.tile as tile
    from concourse import bacc, library_config
    from contextlib import ExitStack

    _patch_dma_gather()

    f16 = mybir.dt.float16
    f32 = mybir.dt.float32
    i16 = mybir.dt.int16
    AL = mybir.AluOpType
    AF = mybir.ActivationFunctionType

    nc = bacc.Bacc(None)

    table_d = nc.declare_dram_parameter("table", [NQ * QR, 128], f16,
                                        isOutput=False)
    idx_d = nc.declare_dram_parameter("idx16", [128, tot16], i16,
                                      isOutput=False)
    aux_d = nc.declare_dram_parameter("aux", [NBLK, 1, 256], f16,
                                      isOutput=False)
    gw2_d = nc.declare_dram_parameter("gw2", [128, 128], f16, isOutput=False)
    dup64_d = nc.declare_dram_parameter("dup64", [64, 128], f16, isOutput=False)
    segf_d = nc.declare_dram_parameter("segf", [128, 64], f16, isOutput=False)
    gseg_d = nc.declare_dram_parameter("gseg", [128, 64], f16, isOutput=False)
    g0t64_d = nc.declare_dram_parameter("g0t64", [64, 64], f16, isOutput=False)
    g1aug_d = nc.declare_dram_parameter("g1aug", [65, 64], f16, isOutput=False)
    l0T_d = nc.declare_dram_parameter("l0T", [64, 64], f16, isOutput=False)
    l1aug_d = nc.declare_dram_parameter("l1aug", [65, 64], f16, isOutput=False)
    i64f_d = nc.declare_dram_parameter("i64f", [64, 64], f32, isOutput=False)
    ident_d = nc.declare_dram_parameter("ident", [128, 128], f16,
                                        isOutput=False)
    ones_d = nc.declare_dram_parameter("ones164", [1, 64], f16, isOutput=False)
    cols_d = nc.declare_dram_parameter("cols", [64, 5], f32, isOutput=False)
    out_d = nc.declare_dram_parameter("out", [BS, D], f32, isOutput=True)

    with tile.TileContext(nc) as tc, ExitStack() as ctx:
        const = ctx.enter_context(tc.tile_pool(name="const", bufs=1))
        sb = ctx.enter_context(tc.tile_pool(name="sb", bufs=2))
        sbf = ctx.enter_context(tc.tile_pool(name="sbf", bufs=2))
        ptr = ctx.enter_context(tc.tile_pool(name="ptr", bufs=2, space="PSUM"))
        pu = ctx.enter_context(tc.tile_pool(name="pu", bufs=2, space="PSUM"))
        pseg = ctx.enter_context(tc.tile_pool(name="pseg", bufs=1, space="PSUM"))
        pracc = ctx.enter_context(
            tc.tile_pool(name="pracc", bufs=1, space="PSUM"))
        psm = ctx.enter_context(tc.tile_pool(name="psm", bufs=2, space="PSUM"))

        nc.gpsimd.load_library(library_config.mlp)

        def cload(dram, shape, dt, tag):
            t = const.tile(shape, dt, tag=tag)
            nc.sync.dma_start(t[:], dram[:])
            return t

        gw2_t = cload(gw2_d, [128, 128], f16, "gw2")
        dup64_t = cload(dup64_d, [64, 128], f16, "dup64")
        segf_t = cload(segf_d, [128, 64], f16, "segf")
        gseg_t = cload(gseg_d, [128, 64], f16, "gseg")
        g0t64_t = cload(g0t64_d, [64, 64], f16, "g0t64")
        g1aug_t = cload(g1aug_d, [65, 64], f16, "g1aug")
        l0T_t = cload(l0T_d, [64, 64], f16, "l0T")
        l1aug_t = cload(l1aug_d, [65, 64], f16, "l1aug")
        i64f_t = cload(i64f_d, [64, 64], f32, "i64f")
        ident_t = cload(ident_d, [128, 128], f16, "ident")
        ones_t = cload(ones_d, [1, 64], f16, "ones")
        cols_t = cload(cols_d, [64, 5], f32, "cols")
        onesrow_t = const.tile([1, 128], f16, tag="onesrow")
        nc.vector.memset(onesrow_t[:], 1.0)
        numw_c = cols_t[:, 0:1]
        numb_c = cols_t[:, 1:2]
        gb0_c = cols_t[:, 2:3]
        lb0_c = cols_t[:, 3:4]
        cc_c = cols_t[:, 4:5]

        for blk in range(NBLK):
            Ks, S, n_c, off = sched[blk]

            idxt = sb.tile([128, S * 8], i16, tag="idx")
            nc.sync.dma_start(idxt[:], idx_d[:, off:off + S * 8])
            aux = sbf.tile([1, 256], f16, tag="aux")
            nc.sync.dma_start(aux[:], aux_d[blk])

            emb = sb.tile([128, S, 64], f16, tag="emb")
            so = 0
            co = 0
            for k in range(NQ):
                Kq = Ks[k]
                if Kq == 0:
                    continue
                nc.gpsimd.dma_gather(
                    emb[:, so:so + Kq, :],
                    table_d[k * QR:(k + 1) * QR, 0:64],
                    idxt[:, co:co + Kq * 8],
                    Kq * 128, Kq * 128, 64, elem_step=128,
                    single_packet=False)
                so += Kq
                co += Kq * 8

            # transposes -> [etT|sq] interleaved chunks
            ev = emb[:].rearrange("p s d -> p (s d)")
            etsq = sb.tile([128, n_c * 256], f16, tag="et")
            etv = etsq[:].rearrange("p (j c) -> p j c", c=256)
            for g0 in range(0, n_c, 8):
                gn = min(8, n_c - g0)
                trp = ptr.tile([128, 1024], f16, tag="tr")
                for j in range(gn):
                    nc.tensor.matmul(
                        out=trp[:, j * 128:(j + 1) * 128],
                        lhsT=ev[:, (g0 + j) * 128:(g0 + j + 1) * 128],
                        rhs=ident_t[:], is_transpose=True,
                        start=True, stop=True)
                nc.vector.tensor_copy(
                    out=etv[:, g0:g0 + gn, 0:128],
                    in_=trp[:, 0:gn * 128].rearrange("p (j c) -> p j c", c=128))
            # squares
            nc.vector.tensor_tensor(
                out=etv[:, :, 128:256], in0=etv[:, :, 0:128],
                in1=etv[:, :, 0:128], op=AL.mult)
            # seg: summedT | sumsqT
            seg = pseg.tile([64, 256], f32, tag="seg")
            for j in range(n_c):
                nc.tensor.matmul(
                    out=seg[:], lhsT=segf_t[:],
                    rhs=etsq[:, j * 256:(j + 1) * 256],
                    start=(j == 0), stop=(j == n_c - 1),
                    skip_group_check=True)
            # numeric embedding (transposed)
            nrep_t = psm.tile([128, 128], f32, tag="small")
            nrep = nrep_t[0:64, :]
            nc.tensor.matmul(out=nrep, lhsT=ones_t[:], rhs=aux[0:1, 0:128],
                             start=True, stop=True)
            numembT = sbf.tile([64, 128], f32, tag="numembT")
            nc.scalar.activation(out=numembT[:], in_=nrep,
                                 func=AF.Identity, bias=numb_c, scale=numw_c)
            # folds
            ssT = sbf.tile([64, 256], f32, tag="ssT")
            nc.vector.tensor_copy(out=ssT[:], in_=seg[:])
            sumT = sbf.tile([64, 128], f32, tag="sumT")
            nc.vector.tensor_tensor(out=sumT[:], in0=ssT[:, 0:128],
                                    in1=numembT[:], op=AL.add)
            # carrier = summed/PROBE + c
            car32 = sbf.tile([64, 128], f32, tag="car32")
            nc.scalar.activation(out=car32[:], in_=sumT[:], func=AF.Identity,
                                 bias=cc_c, scale=1.0 / PROBE)
            car16 = sbf.tile([64, 128], f16, tag="car16")
            nc.vector.tensor_copy(out=car16[:], in_=car32[:])
            pcar = psm.tile([128, 128], f32, tag="small")
            nc.tensor.matmul(out=pcar[:], lhsT=dup64_t[:], rhs=car16[:],
                             start=True, stop=True)
            car2 = sbf.tile([128, 128], f16, tag="car2")
            nc.vector.tensor_copy(out=car2[:], in_=pcar[:])
            # z-add in place on et cols
            nc.vector.tensor_tensor(
                out=etv[:, :, 0:128], in0=etv[:, :, 0:128],
                in1=car2[:].rearrange("p (o n) -> p o n", o=1)
                .to_broadcast([128, n_c, 128]),
                op=AL.add)
            # u matmuls + relu evict
            r16 = sb.tile([128, n_c * 128], f16, tag="r16")
            for g0 in range(0, n_c, 4):
                gn = min(4, n_c - g0)
                up = pu.tile([128, 512], f32, tag="u")
                for j in range(gn):
                    nc.tensor.matmul(
                        out=up[:, j * 128:(j + 1) * 128], lhsT=gw2_t[:],
                        rhs=etv[:, g0 + j:g0 + j + 1, 0:128], start=True, stop=True)
                nc.scalar.activation(
                    out=r16[:, g0 * 128:(g0 + gn) * 128],
                    in_=up[:, 0:gn * 128], func=AF.Relu)
            # numeric field u
            znum = sbf.tile([64, 128], f16, tag="znum")
            nc.vector.tensor_tensor(out=znum[:], in0=numembT[:],
                                    in1=car32[:], op=AL.add)
            unum_t = psm.tile([128, 128], f32, tag="small")
            unum = unum_t[0:64, :]
            nc.tensor.matmul(out=unum, lhsT=gw2_t[0:64, 0:64],
                             rhs=znum[:], start=True, stop=True)
            rnum16 = sbf.tile([64, 128], f16, tag="rnum16")
            nc.scalar.activation(out=rnum16[:], in_=unum, func=AF.Relu)
            # pad correction: -npad * relu(t),  t = carrier @ W1
            pt_t = psm.tile([128, 128], f32, tag="small")
            pt = pt_t[0:64, :]
            nc.tensor.matmul(out=pt, lhsT=gw2_t[0:64, 0:64],
                             rhs=car16[:], start=True, stop=True)
            tr16 = sbf.tile([64, 128], f16, tag="tr16")
            nc.scalar.activation(out=tr16[:], in_=pt, func=AF.Relu)
            pn_t = psm.tile([128, 128], f32, tag="small")
            pn = pn_t[0:64, :]
            nc.tensor.matmul(out=pn, lhsT=ones_t[:], rhs=aux[0:1, 128:256],
                             start=True, stop=True)
            npad64 = sbf.tile([64, 128], f16, tag="npad64")
            nc.vector.tensor_copy(out=npad64[:], in_=pn)
            trs16 = sbf.tile([64, 128], f16, tag="trs16")
            nc.vector.tensor_tensor(out=trs16[:], in0=tr16[:],
                                    in1=npad64[:], op=AL.mult)
            # racc: g1^T accumulation (gW0/NF folded into lhsT)
            gacc = pracc.tile([64, 128], f32, tag="racc")
            for j in range(n_c):
                nc.tensor.matmul(
                    out=gacc[:], lhsT=gseg_t[:],
                    rhs=r16[:, j * 128:(j + 1) * 128],
                    start=(j == 0), stop=False, skip_group_check=True)
            nc.tensor.matmul(out=gacc[:], lhsT=g0t64_t[:], rhs=rnum16[:],
                             start=False, stop=False, skip_group_check=True)
            nc.tensor.matmul(out=gacc[:], lhsT=g0t64_t[:], rhs=trs16[:],
                             start=False, stop=True, skip_group_check=True)
            h1aug = sbf.tile([65, 128], f16, tag="h1aug")
            nc.scalar.activation(out=h1aug[0:64, :], in_=gacc[:],
                                 func=AF.Relu, bias=gb0_c)
            nc.vector.tensor_copy(out=h1aug[64:65, :], in_=onesrow_t[:])
            # local branch
            lsq = sbf.tile([64, 128], f32, tag="lsq")
            nc.vector.tensor_tensor(out=lsq[:], in0=sumT[:], in1=sumT[:],
                                    op=AL.mult)
            lT16 = sbf.tile([64, 128], f16, tag="lT16")
            nc.vector.tensor_tensor(out=lT16[:], in0=lsq[:],
                                    in1=ssT[:, 128:256], op=AL.subtract)
            l1p_t = psm.tile([128, 128], f32, tag="small")
            l1p = l1p_t[0:64, :]
            nc.tensor.matmul(out=l1p, lhsT=l0T_t[:], rhs=lT16[:],
                             start=True, stop=True)
            l1aug = sbf.tile([65, 128], f16, tag="l1aug")
            nc.scalar.activation(out=l1aug[0:64, :], in_=l1p,
                                 func=AF.Relu, bias=lb0_c)
            nc.vector.tensor_copy(out=l1aug[64:65, :], in_=onesrow_t[:])
            # combine + final transpose
            outp_t = psm.tile([128, 128], f32, tag="small")
            outp = outp_t[0:64, :]
            nc.tensor.matmul(out=outp, lhsT=g1aug_t[:], rhs=h1aug[:],
                             start=True, stop=False, skip_group_check=True)
            nc.tensor.matmul(out=outp, lhsT=l1aug_t[:], rhs=l1aug[:],
                             start=False, stop=True, skip_group_check=True)
            outT = sbf.tile([64, 128], f32, tag="outT")
            nc.vector.tensor_copy(out=outT[:], in_=outp)
            finp_t = psm.tile([128, 128], f32, tag="small")
            finp = finp_t[:, 0:64]
            nc.tensor.matmul(out=finp, lhsT=outT[:], rhs=i64f_t[:],
                             is_transpose=True, start=True, stop=True)
            orow = sbf.tile([128, 64], f32, tag="orow")
            nc.vector.tensor_copy(out=orow[:], in_=finp)
            nc.sync.dma_start(out_d[blk * BLK:(blk + 1) * BLK, :], orow[:])

    return nc


def _get_nc(sched, tot16):
    key = ("nc", tuple((tuple(s[0]), s[1], s[2], s[3]) for s in sched))
    if _CACHE.get("key") != key:
        print("[kernel] building bass module...", flush=True)
        nc = _build(sched, tot16)
        nc.finalize()
        _CACHE["nc"] = nc
        _CACHE["key"] = key
        print("[kernel] build done", flush=True)
    return _CACHE["nc"]


def kernel(cat_indices, num_features, embed_table, num_W, num_b,
           ga_W, ga_b, gW, gb, lW, lb):
    from concourse.bass_utils import run_bass_kernel_spmd

    inputs = dict(
        cat_indices=cat_indices, num_features=num_features,
        embed_table=embed_table, num_W=num_W, num_b=num_b,
        ga_W=ga_W, ga_b=ga_b, gW=gW, gb=gb, lW=lW, lb=lb)
    in_maps, sched, order = _prepare(inputs)
    tot16 = in_maps[0]["idx16"].shape[1]
    nc = _get_nc(sched, tot16)

    print("[kernel] launching spmd run...", flush=True)
    res = run_bass_kernel_spmd(nc, in_maps, list(range(NCORES)))
    print("[kernel] run complete", flush=True)

    out = np.empty((B, D), np.float32)
    rows = np.arange(BS)
    base = (rows // BLK) * (NCORES * BLK) + (rows % BLK) * NCORES
    for core in range(NCORES):
        out[order[base + core]] = np.asarray(
            res.results[core]["out"]).astype(np.float32)
    return out
